# revision 1
# baseline (speedup 1.0000x reference)
"""Trainium2 Bass kernel for multi-head attention + output projection.

Problem: B=4, N=2048, D=512, H=8 heads (head_dim 64), TEMP=8.0.
  logits = (Q @ K^T) / TEMP per head; P = softmax(logits); out = P @ V
  final = concat_heads(out) @ W_comb.T + b_comb

Sharding: 8 cores = 4 batches x 2 query-halves. Each core computes a full
(1024, 512) output slab independently (keys/values replicated per batch);
no collectives. Gather = pure reshape on host. Q, K and W are passed to
each core PRE-TRANSPOSED (d-major) -- a host-side layout choice that lets
every on-chip matmul read its operands directly with large contiguous DMAs
and no on-chip transposes.

Per-core algorithm, float32r matmuls (fp32 bit layout, TensorE reduced
mode: 4x faster than fp32, ~1.5e-4 matmul rel err; inputs declared f32r so
HWDGE loads them without casts), "transposed attention" so the PV matmul
needs no transpose of the softmax matrix:
  S^T[k, q] = K_h @ Q_h^T  (stationary = K^T tile, moving = Q^T; the head
              pair packs the 128 contraction rows -> row-tiled concurrent
              matmuls at base partitions 0 / 64)
  E^T = exp(S^T / TEMP)    (ScalarE straight from PSUM, f32r out; no
              max-subtraction: logits ~ N(0,1), exp is fp32-safe)
  O^T_ext = V_ext^T @ E^T  (stationary = V tile with a ones column at index
              64+32*hh, so partition 64/96 of the PSUM accumulator becomes
              the softmax denominator; 32-aligned so DVE can slice it)
  O = O^T / denom          (per-head: reciprocal on a DMA-reshaped [64,16]
              tile -> 16 elems/lane; DMA partition-broadcast via DRAM
              scratch; one in-place tensor_mul)
  F += O_h^T.T @ W^T_h     (incremental per pair into SBUF accumulators,
              bias folded into the first pair's add)

Schedule shaping (Tile scheduler follows emission order per engine): pair
p's kt-loop carries, interleaved, the previous pair's projection (late, at
kt = 11/13/15, after the previous pair's normalization chain has surely
drained, so its PSUM slot steals land where ACT has slack) and the next
pair's loads (kt==10). The last pair's normalization broadcasts its
reciprocals with a ones-row matmul through idle PE/PSUM instead of the
DRAM round-trip.
"""

import numpy as np

import concourse.bass as bass
import concourse.mybir as mybir
from concourse.tile import TileContext

F32 = mybir.dt.float32
F32R = mybir.dt.float32r

B, N, D, H = 4, 2048, 512, 8
HEAD = 64
TEMP = 8.0
NQ = N // 2          # queries per core
NCORES = 8
NKT = N // 128       # 16 key tiles of 128
NQT = NQ // 128      # 8 query tiles of 128
NPAIR = H // 2       # 4 head pairs

# this walrus build encodes at most 1 sync-wait per instruction
_MAX_WAITS = 1


def _split_excess_waits(nc):
    """Move excess per-instruction sem-waits onto preceding NoOps."""
    n_split = 0
    for f in nc.m.functions:
        for blk in f.blocks:
            insts = blk.instructions
            i = 0
            while i < len(insts):
                inst = insts[i]
                si = getattr(inst, "sync_info", None)
                if si is not None and si.on_wait and len(si.on_wait) > _MAX_WAITS:
                    waits = list(si.on_wait)
                    si.on_wait = waits[:_MAX_WAITS]
                    extra = waits[_MAX_WAITS:]
                    new_insts = []
                    for j in range(0, len(extra), _MAX_WAITS):
                        chunk = extra[j : j + _MAX_WAITS]
                        nop = mybir.InstNoOp(
                            name=f"{inst.name}-waitsplit-{j}",
                            engine=inst.engine,
                            ins=[],
                            outs=[],
                            sync_info=mybir.SyncInfo(on_wait=chunk, on_update=[]),
                        )
                        new_insts.append(nop)
                    insts[i:i] = new_insts
                    i += len(new_insts)
                    n_split += 1
                i += 1
    return n_split


def _build():
    nc = bass.Bass()
    # q/k/w arrive pre-transposed (d-major) from the host sharding step.
    # All matmul operands are declared f32r (same bit layout as fp32) so
    # HWDGE loads them directly; the PE rounds on operand load.
    qt_d = nc.dram_tensor("qt", [D, NQ], F32R, kind="ExternalInput")
    kt_d = nc.dram_tensor("kt", [D, N], F32R, kind="ExternalInput")
    v = nc.dram_tensor("v", [N, D], F32R, kind="ExternalInput")
    wt_d = nc.dram_tensor("wt", [D, D], F32R, kind="ExternalInput")
    bvec = nc.dram_tensor("bvec", [D], F32, kind="ExternalInput")
    out = nc.dram_tensor("out", [NQ, D], F32, kind="ExternalOutput")
    recips_dram = nc.dram_tensor("recips_scratch", [H, 1024], F32, kind="Internal")

    v_r = v[:, :].rearrange("(a i) d -> i a d", i=128)  # [128, 16, 512]

    with TileContext(nc) as tc:
        with (
            tc.tile_pool(name="singles", bufs=1) as singles,
            tc.tile_pool(name="tp", bufs=2) as tp,
            tc.tile_pool(name="epool", bufs=8) as epool,
            tc.tile_pool(name="nrm", bufs=2) as nrm,
            tc.tile_pool(name="psum_s", bufs=2, space="PSUM") as psum_s,
            tc.tile_pool(name="psum_o", bufs=2, space="PSUM") as psum_o,
        ):
            bias_bc = singles.tile([128, D], F32)

            # per-head O^T + denominator: rows 0:64 = O^T (normalized in
            # place), row 64 (even head) / 96 (odd head) = denominator
            otmp = []
            wts = []    # per-head W^T tiles [64 d_in, 512 d_out]
            fsb = []    # output accumulators [128 q, 512]
            for h in range(H):
                rows = 65 if h % 2 == 0 else 97
                t = singles.tile([rows, 1024], F32R, name=f"otmp{h}", tag=f"otmp{h}")
                otmp.append(t)
                t = singles.tile([64, D], F32R, name=f"wt{h}", tag=f"wt{h}")
                wts.append(t)
            for i in range(NQT):
                t = singles.tile([128, D], F32, name=f"fsb{i}", tag=f"fsb{i}")
                fsb.append(t)

            # fp32 staging for the f32r zero/one columns of V_ext
            vstage = singles.tile([128, NKT, 33], F32)
            nc.vector.memset(vstage[:, :, 0:32], 0.0)
            nc.vector.memset(vstage[:, :, 32:33], 1.0)
            ones_f = singles.tile([1, 64], F32)
            nc.vector.memset(ones_f, 1.0)
            ones_row = singles.tile([1, 64], F32R)
            nc.gpsimd.dma_start(out=ones_row, in_=ones_f)

            # persistent double-buffered V_ext tiles; the zero/ones columns
            # are written once, the V data is re-DMA'd every pair
            vxt = {0: [], 1: []}
            for hh in range(2):
                ocol = 64 + 32 * hh
                for j in range(2):
                    vx = singles.tile(
                        [128, NKT, ocol + 1], F32R,
                        name=f"vxt{hh}_{j}", tag=f"vxt{hh}_{j}",
                    )
                    vxt[hh].append(vx)

            def emit_vxt_init(j):
                for hh in range(2):
                    ocol = 64 + 32 * hh
                    vx = vxt[hh][j]
                    if hh == 1:
                        nc.gpsimd.dma_start(
                            out=vx[:, :, 64:97], in_=vstage[:, :, 0:33]
                        )
                    else:
                        nc.gpsimd.dma_start(
                            out=vx[:, :, 64:65], in_=vstage[:, :, 32:33]
                        )

            def emit_pair_loads(p):
                """Issue DMA loads for pair p; returns (qt, kt_sb, vext)."""
                hA, hB = 2 * p, 2 * p + 1
                qt = tp.tile([128, NQ], F32R, name=f"qt{p}", tag="qt")
                nc.sync.dma_start(out=qt, in_=qt_d[p * 128 : (p + 1) * 128, :])
                kt_sb = tp.tile([128, N], F32R, name=f"ktile{p}", tag="ktile")
                nc.sync.dma_start(
                    out=kt_sb[:, 0:256], in_=kt_d[p * 128 : (p + 1) * 128, 0:256]
                )
                nc.sync.dma_start(
                    out=kt_sb[:, 256:1024], in_=kt_d[p * 128 : (p + 1) * 128, 256:1024]
                )
                vext = {}
                vA = vxt[0][p % 2]
                nc.sync.dma_start(
                    out=vA[:, :, 0:64], in_=v_r[:, :, hA * HEAD : (hA + 1) * HEAD]
                )
                vext[0] = vA
                nc.sync.dma_start(
                    out=kt_sb[:, 1024:2048],
                    in_=kt_d[p * 128 : (p + 1) * 128, 1024:2048],
                )
                vB = vxt[1][p % 2]
                nc.sync.dma_start(
                    out=vB[:, :, 0:64], in_=v_r[:, :, hB * HEAD : (hB + 1) * HEAD]
                )
                vext[1] = vB
                return qt, kt_sb, vext

            def emit_proj_part(p, tiles, pool=None, tag="ps"):
                """Accumulate pair p's head contributions into fsb[tiles]."""
                hA, hB = 2 * p, 2 * p + 1
                pool = pool or psum_s
                for i in tiles:
                    ps = pool.tile([128, 512], F32, name=f"f{p}_{i}", tag=tag)
                    nc.tensor.matmul(
                        ps,
                        lhsT=otmp[hA][0:64, i * 128 : (i + 1) * 128],
                        rhs=wts[hA],
                        start=True,
                        stop=False,
                    )
                    nc.tensor.matmul(
                        ps,
                        lhsT=otmp[hB][0:64, i * 128 : (i + 1) * 128],
                        rhs=wts[hB],
                        start=False,
                        stop=True,
                    )
                    if p == 0:
                        nc.vector.tensor_add(out=fsb[i], in0=ps, in1=bias_bc)
                    else:
                        nc.vector.tensor_add(out=fsb[i], in0=ps, in1=fsb[i])
                    if p == NPAIR - 1:
                        nc.sync.dma_start(
                            out=out[i * 128 : (i + 1) * 128, :], in_=fsb[i]
                        )

            def emit_norm_head(h, hh, o_ps, tail=False):
                """Drain one head's o_ps, reciprocal its denominator, normalize."""
                rows = 65 if hh == 0 else 97
                drow = 64 + 32 * hh
                if tail and hh == 1:
                    # ACT is idle after its last exp: drain head B there so
                    # both heads' denominator chains start in parallel
                    nc.scalar.copy(otmp[h][0:rows, :], o_ps[hh][0:rows, :])
                else:
                    nc.vector.tensor_copy(otmp[h][0:rows, :], o_ps[hh][0:rows, :])
                dsq = nrm.tile([64, 16], F32, name=f"dsq{h}", tag=f"dsq{hh}")
                nc.sync.dma_start(
                    out=dsq, in_=otmp[h][drow : drow + 1, :].bitcast(F32)
                )
                rsq = nrm.tile([64, 16], F32, name=f"rsq{h}", tag=f"rsq{hh}")
                nc.vector.reciprocal(rsq, dsq)
                if tail:
                    # PE/PSUM are idle at the tail: broadcast via a ones-row
                    # matmul instead of the DRAM round-trip (saves a DMA hop)
                    strip = nrm.tile([1, 1024], F32R, name=f"strip{h}", tag=f"st{hh}")
                    nc.gpsimd.dma_start(out=strip, in_=rsq)
                    rbp = psum_s.tile([64, 1024], F32, name=f"rbp{h}", tag="ps")
                    for qc in range(2):
                        nc.tensor.matmul(
                            rbp[:, qc * 512 : (qc + 1) * 512],
                            lhsT=ones_row,
                            rhs=strip[:, qc * 512 : (qc + 1) * 512],
                            start=True,
                            stop=True,
                        )
                    nc.vector.tensor_mul(otmp[h][0:64, :], otmp[h][0:64, :], rbp)
                    return
                nc.sync.dma_start(out=recips_dram[h : h + 1, :], in_=rsq)
                rbc = nrm.tile([64, 1024], F32, name=f"rbc{h}", tag=f"rbc{hh}")
                nc.sync.dma_start(
                    out=rbc,
                    in_=recips_dram[h : h + 1, :].partition_broadcast(64),
                )
                nc.vector.tensor_mul(otmp[h][0:64, :], otmp[h][0:64, :], rbc)

            nxt = emit_pair_loads(0)
            emit_vxt_init(0)
            for p in range(NPAIR):
                hA, hB = 2 * p, 2 * p + 1
                qt, kt_sb, vext = nxt

                o_ps = {
                    0: psum_o.tile([65, 1024], F32, name=f"o{hA}", tag="o"),
                    1: psum_o.tile([97, 1024], F32, name=f"o{hB}", tag="o"),
                }

                for kt in range(NKT):
                    if p == 0 and kt == 2:
                        nc.gpsimd.dma_start(
                            out=bias_bc, in_=bvec[:].partition_broadcast(128)
                        )
                    if p == 0 and kt == 6:
                        emit_vxt_init(1)
                    if p == 0 and kt == 8:
                        for h in range(H):
                            nc.sync.dma_start(
                                out=wts[h], in_=wt_d[h * HEAD : (h + 1) * HEAD, :]
                            )
                    if kt == 10 and p + 1 < NPAIR:
                        nxt = emit_pair_loads(p + 1)

                    if p > 0 and kt in (11, 13, 15):
                        emit_proj_part(
                            p - 1,
                            ((kt - 11) // 2, (kt - 11) // 2 + 3)
                            if kt < 15
                            else (2, 5, 6, 7),
                        )
                    for hh, h in ((0, hA), (1, hB)):
                        base = hh * 64
                        s_ps = psum_s.tile(
                            [128, 1024], F32, name=f"s{h}_{kt}", tag="ps"
                        )
                        for qc in range(2):
                            nc.tensor.matmul(
                                s_ps[:, qc * 512 : (qc + 1) * 512],
                                lhsT=kt_sb[base : base + 64, kt * 128 : (kt + 1) * 128],
                                rhs=qt[base : base + 64, qc * 512 : (qc + 1) * 512],
                                start=True,
                                stop=True,
                            )
                        e_sb = epool.tile(
                            [128, 1024], F32R, name=f"e{h}_{kt}", tag="e"
                        )
                        nc.scalar.activation(
                            e_sb,
                            s_ps,
                            mybir.ActivationFunctionType.Exp,
                            bias=0.0,
                            scale=1.0 / TEMP,
                        )
                        for qc in range(2):
                            nc.tensor.matmul(
                                o_ps[hh][:, qc * 512 : (qc + 1) * 512],
                                lhsT=vext[hh][:, kt, :],
                                rhs=e_sb[:, qc * 512 : (qc + 1) * 512],
                                start=(kt == 0),
                                stop=(kt == NKT - 1),
                            )

                tail = p == NPAIR - 1
                emit_norm_head(hA, 0, o_ps, tail=tail)
                emit_norm_head(hB, 1, o_ps, tail=tail)

            emit_proj_part(NPAIR - 1, range(NQT))

    _split_excess_waits(nc)
    return nc


_NC_CACHE = {}


def _get_nc():
    if "nc" not in _NC_CACHE:
        _NC_CACHE["nc"] = _build()
    return _NC_CACHE["nc"]


def kernel(keys, queries, values, W_comb, b_comb, _collect=None):
    from concourse.bass_utils import run_bass_kernel_spmd

    keys = np.ascontiguousarray(keys, dtype=np.float32)
    queries = np.ascontiguousarray(queries, dtype=np.float32)
    values = np.ascontiguousarray(values, dtype=np.float32)
    W_comb = np.ascontiguousarray(W_comb, dtype=np.float32)
    b_comb = np.ascontiguousarray(b_comb, dtype=np.float32)

    nc = _get_nc()
    wt_np = np.ascontiguousarray(W_comb.T)
    in_maps = []
    for c in range(NCORES):
        b, half = divmod(c, 2)
        in_maps.append(
            {
                "qt": np.ascontiguousarray(
                    queries[b, half * NQ : (half + 1) * NQ, :].T
                ),
                "kt": np.ascontiguousarray(keys[b].T),
                "v": values[b],
                "wt": wt_np,
                "bvec": b_comb,
            }
        )
    kwargs = dict(_collect) if _collect else {}
    res = run_bass_kernel_spmd(nc, in_maps, core_ids=list(range(NCORES)), **kwargs)

    full = np.empty((B, N, D), dtype=np.float32)
    for c, r in enumerate(res.results):
        b, half = divmod(c, 2)
        full[b, half * NQ : (half + 1) * NQ, :] = r["out"]
    if _collect is not None:
        return full, res
    return full



# revision 21
# speedup vs baseline: 1.1832x; 1.1832x over previous
"""Trainium2 Bass kernel for multi-head attention + output projection.

Problem: B=4, N=2048, D=512, H=8 heads (head_dim 64), TEMP=8.0.
  logits = (Q @ K^T) / TEMP per head; P = softmax(logits); out = P @ V
  final = concat_heads(out) @ W_comb.T + b_comb

Sharding: 8 cores = 4 batches x 2 query-halves; each core computes a full
(1024, 512) output slab; gather is a host reshape. Q, K, W arrive
pre-transposed (d-major); V arrives fp16.

Per-core pipeline ("transposed attention": S^T = K @ Q^T per head so the
PV matmul needs no transposes; the softmax denominator rides the PV
matmul as a fp16 ones-column at stationary col 64 -> PSUM row 64):

  Flat stream over (pair, kt) steps (4 head pairs x 16 key tiles). Per
  step: S^T matmuls (f32r, two heads row-packed on the PE array), then
  exp -> E (fp16) on EITHER ScalarE (ACT) or, for 6 tiles per pair, a
  7-op integer/poly chain on the Vector engine (DVE):
     w  = int16(s * 1024*log2e/8 + 1024*15)      (tensor_scalar)
     hi = w & ~1023                              (exponent bits)
     g  = (w & 1023) | 0x3C00 -> fp16 in [1,2)   (mantissa bits)
     E  = (C2*g^2 + B2*g + A2) * hi.bitcast(f16) (quadratic 2^(g-1))
  This splits exp across two engines: ACT streams 1 elem/lane/cycle and
  is the critical resource; DVE runs the 2-byte ops at 2-4x rate. PV
  matmuls lag S by 1 step (head A) / 2 steps (head B) so the PE never
  waits on ACT/DVE; DVE-offloaded tiles' PVs lag 2 steps further.

  At kt==15 of each pair the pair is flushed: remaining PVs (stop=True
  last), then O^T rows and denominators drain from PSUM by DMA into a
  packed [128, 1024] SBUF tile per pair (head B partition-shifted to
  rows 64:127) -- freeing the PSUM accumulators just in time for the
  next pair's first PV. Reciprocals broadcast via a DRAM round-trip and
  the normalization multiplies run on the otherwise-idle GPSIMD (Pool)
  engine. The projection runs entirely in the tail: per 128-query tile,
  4 accumulating K=128 matmuls (one per packed pair) + bias add + store.
  The last pair normalizes via a row-reciprocal off the PSUM denominator
  row + ones-row matmul broadcast, skipping the DRAM round-trip.
"""

import numpy as np

import concourse.bass as bass
import concourse.mybir as mybir
from concourse.tile import TileContext

F32 = mybir.dt.float32
F32R = mybir.dt.float32r
F16 = mybir.dt.float16
I16 = mybir.dt.int16
ALU = mybir.AluOpType

B, N, D, H = 4, 2048, 512, 8
HEAD = 64
TEMP = 8.0
NQ = N // 2          # queries per core
NCORES = 8
NKT = N // 128       # 16 key tiles of 128
NQT = NQ // 128      # 8 query tiles of 128
NPAIR = H // 2       # 4 head pairs

# fast-exp chain constants: quadratic fit of 2^(g-1) on g in [1,2)
_A, _B, _C = 1.00136022, 0.64677132, 0.35065519
A2 = _A - _B + _C
B2 = _B - 2 * _C
C2 = _C
C1 = float(1024.0 * np.log2(np.e) / TEMP)
K1 = float(1024.0 * 15.0)

# pair-local kt whose exp runs on DVE instead of ACT, per head
OFF_A = (2, 6, 10)   # head A (hh=0)
OFF_B = (4, 12)      # head B (hh=1)

# this walrus build encodes at most 1 sync-wait per instruction
_MAX_WAITS = 1


def _split_excess_waits(nc):
    """Move excess per-instruction sem-waits onto preceding NoOps."""
    n_split = 0
    for f in nc.m.functions:
        for blk in f.blocks:
            insts = blk.instructions
            i = 0
            while i < len(insts):
                inst = insts[i]
                si = getattr(inst, "sync_info", None)
                if si is not None and si.on_wait and len(si.on_wait) > _MAX_WAITS:
                    waits = list(si.on_wait)
                    si.on_wait = waits[:_MAX_WAITS]
                    extra = waits[_MAX_WAITS:]
                    new_insts = []
                    for j in range(0, len(extra), _MAX_WAITS):
                        chunk = extra[j : j + _MAX_WAITS]
                        nop = mybir.InstNoOp(
                            name=f"{inst.name}-waitsplit-{j}",
                            engine=inst.engine,
                            ins=[],
                            outs=[],
                            sync_info=mybir.SyncInfo(on_wait=chunk, on_update=[]),
                        )
                        new_insts.append(nop)
                    insts[i:i] = new_insts
                    i += len(new_insts)
                    n_split += 1
                i += 1
    return n_split


def _build():
    nc = bass.Bass()
    qt_d = nc.dram_tensor("qt", [D, NQ], F32R, kind="ExternalInput")
    kt_d = nc.dram_tensor("kt", [D, N], F32R, kind="ExternalInput")
    v = nc.dram_tensor("v", [N, D], F16, kind="ExternalInput")
    wt_d = nc.dram_tensor("wt", [D, D], F32R, kind="ExternalInput")
    bvec = nc.dram_tensor("bvec", [D], F32, kind="ExternalInput")
    out = nc.dram_tensor("out", [NQ, D], F32, kind="ExternalOutput")
    recips_dram = nc.dram_tensor("recips_scratch", [H, 1024], F32, kind="Internal")

    v_r = v[:, :].rearrange("(a i) d -> i a d", i=128)  # [128, 16, 512] f16

    with nc.allow_low_precision(
        reason="f32r tiles share fp32 bit layout; all matmul accumulation "
        "is fp32 in PSUM, f32r is only the SBUF storage dtype"
    ), TileContext(nc) as tc:
        with (
            tc.tile_pool(name="singles", bufs=1) as singles,
            tc.tile_pool(name="tp", bufs=2) as tp,
            tc.tile_pool(name="epool", bufs=9) as epool,
            tc.tile_pool(name="chain", bufs=2) as chain,
            tc.tile_pool(name="nrm", bufs=2) as nrm,
            tc.tile_pool(name="psum_s", bufs=2, space="PSUM") as psum_s,
            tc.tile_pool(name="psum_o", bufs=2, space="PSUM") as psum_o,
        ):
            bias_bc = singles.tile([128, D], F32)

            # packed per-pair normalized O^T: rows 0:64 head A, 64:128 head B
            otmp = [
                singles.tile([128, 1024], F32R, name=f"otmp{p}", tag=f"otmp{p}")
                for p in range(NPAIR)
            ]
            # W^T slices: packed [128 d_in, 512] for pairs 0-2; the last
            # pair stays unpacked (two [64, 512] tiles at partitions 0:64)
            wtsp = [
                singles.tile([128, D], F32R, name=f"wtsp{p}", tag=f"wtsp{p}")
                for p in range(NPAIR - 1)
            ]
            w3 = [
                singles.tile([64, D], F32R, name=f"w3{hh}", tag=f"w3{hh}")
                for hh in range(2)
            ]
            fsb = [
                singles.tile([128, D], F32, name=f"fsb{i}", tag=f"fsb{i}")
                for i in range(NQT)
            ]

            # fp16 ones staging for the V_ext ones column; f32 ones row for
            # the tail broadcast matmul (read .bitcast(F32R), lane 64)
            vstage = singles.tile([128, NKT, 1], F16)
            nc.vector.memset(vstage, 1.0)
            ones65f = singles.tile([65, 64], F32)
            nc.vector.memset(ones65f, 1.0)
            ones65 = singles.tile([65, 64], F32R)
            nc.gpsimd.dma_start(out=ones65, in_=ones65f)

            # persistent double-buffered V_ext tiles [128, NKT, 65] fp16:
            # cols 0:64 = V head slice (re-DMA'd per pair), col 64 = ones
            vxt = {0: [], 1: []}
            for hh in range(2):
                for j in range(2):
                    vx = singles.tile(
                        [128, NKT, HEAD + 1], F16,
                        name=f"vxt{hh}_{j}", tag=f"vxt{hh}_{j}",
                    )
                    vxt[hh].append(vx)

            def emit_vxt_init(j):
                for hh in range(2):
                    nc.gpsimd.dma_start(
                        out=vxt[hh][j][:, :, HEAD : HEAD + 1], in_=vstage
                    )

            def emit_pair_loads(p, first=False):
                hA, hB = 2 * p, 2 * p + 1
                qt = tp.tile([128, NQ], F32R, name=f"qt{p}", tag="qt")
                kt_sb = tp.tile([128, N], F32R, name=f"ktile{p}", tag="ktile")
                if first:
                    # parallel queues at the cold start: K tiles on the SP
                    # queue (fine-grained so chunk k beats step k), Q on the
                    # ACT queue (idle until the first exp)
                    nc.scalar.dma_start(
                        out=qt[:, 0:512], in_=qt_d[p * 128 : (p + 1) * 128, 0:512]
                    )
                    nc.scalar.dma_start(
                        out=qt[:, 512:1024], in_=qt_d[p * 128 : (p + 1) * 128, 512:1024]
                    )
                    for c0, c1 in ((0, 256), (256, 512), (512, 1024)):
                        nc.sync.dma_start(
                            out=kt_sb[:, c0:c1], in_=kt_d[p * 128 : (p + 1) * 128, c0:c1]
                        )
                else:
                    nc.sync.dma_start(out=qt, in_=qt_d[p * 128 : (p + 1) * 128, :])
                    nc.sync.dma_start(
                        out=kt_sb[:, 0:1024], in_=kt_d[p * 128 : (p + 1) * 128, 0:1024]
                    )
                vA = vxt[0][p % 2]
                nc.sync.dma_start(
                    out=vA[:, :, 0:HEAD], in_=v_r[:, :, hA * HEAD : (hA + 1) * HEAD]
                )
                nc.sync.dma_start(
                    out=kt_sb[:, 1024:2048],
                    in_=kt_d[p * 128 : (p + 1) * 128, 1024:2048],
                )
                vB = vxt[1][p % 2]
                nc.sync.dma_start(
                    out=vB[:, :, 0:HEAD], in_=v_r[:, :, hB * HEAD : (hB + 1) * HEAD]
                )
                return qt, kt_sb, {0: vA, 1: vB}

            def emit_chain_head(s_ps, tag):
                """Op 1 of the DVE fast-exp: read PSUM, release the s slot
                as early as possible."""
                w = chain.tile([128, 1024], I16, name=f"w{tag}", tag="cw")
                nc.vector.tensor_scalar(
                    out=w, in0=s_ps, scalar1=C1, scalar2=K1, op0=ALU.mult, op1=ALU.add
                )
                return w

            def emit_chain_tail(w, e_sb, tag):
                """Ops 2-7 of the DVE fast-exp: e_sb(f16) = exp(./TEMP)."""
                hi = chain.tile([128, 1024], I16, name=f"hi{tag}", tag="chi")
                nc.vector.tensor_scalar(
                    out=hi, in0=w, scalar1=-1024.0, scalar2=0.0,
                    op0=ALU.bitwise_and, op1=ALU.bitwise_or,
                )
                g = chain.tile([128, 1024], I16, name=f"g{tag}", tag="cg")
                nc.vector.tensor_scalar(
                    out=g, in0=w, scalar1=1023.0, scalar2=15360.0,
                    op0=ALU.bitwise_and, op1=ALU.bitwise_or,
                )
                gf = g.bitcast(F16)
                t = chain.tile([128, 1024], F16, name=f"t{tag}", tag="ct")
                nc.vector.tensor_scalar(
                    out=t, in0=gf, scalar1=C2, scalar2=B2, op0=ALU.mult, op1=ALU.add
                )
                pp = chain.tile([128, 1024], F16, name=f"p{tag}", tag="cp")
                nc.vector.tensor_tensor(out=pp, in0=t, in1=gf, op=ALU.mult)
                q = chain.tile([128, 1024], F16, name=f"q{tag}", tag="cq")
                nc.vector.tensor_scalar(
                    out=q, in0=pp, scalar1=A2, scalar2=0.0, op0=ALU.add, op1=ALU.add
                )
                nc.vector.tensor_tensor(
                    out=e_sb, in0=q, in1=hi.bitcast(F16), op=ALU.mult
                )

            # ---------------- flat (pair, kt) pipeline ----------------
            state = {}

            def pair_state(p):
                if p not in state:
                    qt, kt_sb, vext = state.pop("nxt")
                    state[p] = {
                        "qt": qt, "kt": kt_sb, "vext": vext,
                        "o": None,
                        "e": {},
                        "pv_done": set(),
                    }
                return state[p]

            def pair_o(p):
                st = pair_state(p)
                if st["o"] is None:
                    st["o"] = {
                        0: psum_o.tile([65, 1024], F32, name=f"oA{p}", tag="o"),
                        1: psum_o.tile([65, 1024], F32, name=f"oB{p}", tag="o"),
                    }
                return st["o"]

            state["nxt"] = emit_pair_loads(0, first=True)
            emit_vxt_init(0)

            s_done = set()
            s_step = [0]
            pending_chains = []

            def flush_chain_tails():
                while pending_chains:
                    w, e_sb, tag = pending_chains.pop(0)
                    emit_chain_tail(w, e_sb, tag)

            def emit_S_exp(p, kt):
                if (p, kt) in s_done:
                    return
                s_done.add((p, kt))
                st = pair_state(p)
                qt, kt_sb = st["qt"], st["kt"]
                for hh in (0, 1):
                    base = hh * 64
                    s_ps = psum_s.tile(
                        [128, 1024], F32, name=f"s{p}_{kt}_{hh}", tag="ps"
                    )
                    for qc in range(2):
                        nc.tensor.matmul(
                            s_ps[:, qc * 512 : (qc + 1) * 512],
                            lhsT=kt_sb[base : base + 64, kt * 128 : (kt + 1) * 128],
                            rhs=qt[base : base + 64, qc * 512 : (qc + 1) * 512],
                            start=True,
                            stop=True,
                        )
                    e_sb = epool.tile(
                        [128, 1024], F16, name=f"e{p}_{kt}_{hh}", tag=f"e{hh}"
                    )
                    if kt in (OFF_A if hh == 0 else OFF_B):
                        w = emit_chain_head(s_ps, tag=f"{p}_{kt}_{hh}")
                        pending_chains.append((w, e_sb, f"{p}_{kt}_{hh}"))
                    else:
                        nc.scalar.activation(
                            e_sb,
                            s_ps,
                            mybir.ActivationFunctionType.Exp,
                            bias=0.0,
                            scale=1.0 / TEMP,
                        )
                    st["e"][(hh, kt)] = e_sb
                    pv_enqueue(s_step[0], p, kt, hh)

            def emit_pv(p, kt, hh):
                st = pair_state(p)
                if (hh, kt) in st["pv_done"]:
                    return
                st["pv_done"].add((hh, kt))
                e_sb = st["e"].pop((hh, kt))
                o_ps = pair_o(p)[hh]
                for qc in range(2):
                    nc.tensor.matmul(
                        o_ps[:, qc * 512 : (qc + 1) * 512],
                        lhsT=st["vext"][hh][:, kt, :],
                        rhs=e_sb[:, qc * 512 : (qc + 1) * 512],
                        start=(kt == 0),
                        stop=(kt == NKT - 1),
                    )

            pv_ready = []   # (ready_step, p, kt, hh) FIFO per emission time

            def pv_enqueue(step, p, kt, hh):
                off = kt in (OFF_A if hh == 0 else OFF_B)
                lag = (4 if off else 1) if hh == 0 else (5 if off else 2)
                if kt < NKT - 1:
                    pv_ready.append((step + lag, p, kt, hh))

            def pv_sched(step):
                """Credit-based PV scheduling: offload steps (where PE is
                the pacing engine) carry 1 PV, full-ACT steps up to 3; the
                backlog drains FIFO. kt=15 is left for the pair flush."""
                p, kt = divmod(step, NKT)
                is_off = kt in OFF_A or kt in OFF_B
                budget = 1 if is_off else 3
                got = []
                i = 0
                while i < len(pv_ready) and len(got) < budget:
                    rs, pp, kk, hh = pv_ready[i]
                    if rs <= step:
                        got.append((pp, kk, hh))
                        pv_ready.pop(i)
                    else:
                        i += 1
                return got

            def emit_pair_flush(p):
                st = pair_state(p)
                pv_ready[:] = [e for e in pv_ready if e[1] != p]
                for hh in (0, 1):
                    for kt in range(NKT):
                        if (hh, kt) not in st["pv_done"]:
                            emit_pv(p, kt, hh)

            def emit_norm_drain(p):
                """Drain pair p's PSUM accumulators: DVE copies (O rows) and
                row-reciprocals (denominator rows) free the o slots; head B
                then partition-shifts to otmp rows 64:127 by an SBUF->SBUF
                DMA (slack path). Order A-copy, A-recip, B-copy, B-recip so
                each o slot frees as early as possible."""
                st = pair_state(p)
                last = p == NPAIR - 1
                for hh in ((1, 0) if last else (0, 1)):
                    o_ps = pair_o(p)[hh]
                    if hh == 0:
                        dst = otmp[p][0:64, :]
                    else:
                        shB = nrm.tile([64, 1024], F32R, name=f"shB{p}", tag="shB")
                        st["shB"] = shB
                        dst = shB[:, :]
                    if last:
                        # ACT is idle in the tail: put the O-row copies there
                        # so DVE only runs the reciprocals + multiplies
                        nc.scalar.copy(dst, o_ps[0:64, :])
                    else:
                        nc.vector.tensor_copy(dst, o_ps[0:64, :])
                    strip = nrm.tile(
                        [65, 1024], F32R, name=f"strip{p}{hh}", tag=f"st{hh}"
                    )
                    nc.vector.reciprocal(strip[64:65, :], o_ps[64:65, :])
                    st[f"strip{hh}"] = strip
                if not last:
                    # pack head B into otmp rows 64:127 (partition shift)
                    nc.sync.dma_start(out=otmp[p][64:128, :], in_=st["shB"])
                    for hh in (0, 1):
                        nc.sync.dma_start(
                            out=recips_dram[2 * p + hh : 2 * p + hh + 1, :],
                            in_=st[f"strip{hh}"][64:65, :].bitcast(F32),
                        )

            def emit_norm_finish(p):
                """DRAM broadcast + Pool multiply (slack path, pairs 0-2)."""
                rbc = nrm.tile([128, 1024], F32, name=f"rbc{p}", tag="rbc")
                for hh in (0, 1):
                    h = 2 * p + hh
                    nc.sync.dma_start(
                        out=rbc[hh * 64 : hh * 64 + 64, :],
                        in_=recips_dram[h : h + 1, :].partition_broadcast(64),
                    )
                for hh in (0, 1):
                    sl = slice(hh * 64, hh * 64 + 64)
                    nc.gpsimd.tensor_tensor(
                        out=otmp[p][sl, :], in0=otmp[p][sl, :], in1=rbc[sl, :],
                        op=ALU.mult,
                    )

            NSTEP = NPAIR * NKT
            for step in range(NSTEP):
                p, kt = divmod(step, NKT)
                if p == 0 and kt == 2:
                    nc.gpsimd.dma_start(
                        out=bias_bc, in_=bvec[:].partition_broadcast(128)
                    )
                if p == 0 and kt == 6:
                    emit_vxt_init(1)
                if p == 0 and kt == 8:
                    for pp in range(NPAIR - 1):
                        nc.sync.dma_start(
                            out=wtsp[pp], in_=wt_d[pp * 128 : (pp + 1) * 128, :]
                        )
                    for hh in range(2):
                        base = (NPAIR - 1) * 128 + hh * 64
                        nc.sync.dma_start(
                            out=w3[hh], in_=wt_d[base : base + 64, :]
                        )
                if kt == 10 and p + 1 < NPAIR:
                    state["nxt"] = emit_pair_loads(p + 1)

                s_step[0] = step
                pending = list(pending_chains)
                pending_chains.clear()
                emit_S_exp(p, kt)
                for w, e_sb, tag in pending:
                    emit_chain_tail(w, e_sb, tag)
                if kt == NKT - 1 and p + 1 < NPAIR:
                    # pull the next pair's first S + exp ahead of this
                    # step's PVs and the flush so ACT stays gapless across
                    # the boundary
                    emit_S_exp(p + 1, 0)
                for pv_p, pv_kt, pv_hh in pv_sched(step):
                    emit_pv(pv_p, pv_kt, pv_hh)

                if kt == NKT - 1:
                    # flush this pair (stop=True PVs last) and start its
                    # PSUM drain so the o slots free before the next pair's
                    # first PVs execute
                    emit_pair_flush(p)
                    emit_norm_drain(p)
                if kt == 6 and p > 0:
                    emit_norm_finish(p - 1)

            flush_chain_tails()
            # ---------------- tail: last pair norm + projection ----------------
            pl = NPAIR - 1
            st = pair_state(pl)
            # last pair stays unpacked: head A normalized in otmp[pl][0:64],
            # head B in its shB scratch (both at partitions 0:64).
            # reciprocal rows already computed by the flush's norm drain;
            # broadcast via ones-row matmuls into the freed o slots.
            rbp = {}
            for hh in (1, 0):
                rb = psum_o.tile([64, 1024], F32, name=f"rbp{hh}", tag="o")
                for qc in range(2):
                    nc.tensor.matmul(
                        rb[:, qc * 512 : (qc + 1) * 512],
                        lhsT=ones65[64:65, :],
                        rhs=st[f"strip{hh}"][64:65, qc * 512 : (qc + 1) * 512],
                        start=True,
                        stop=True,
                    )
                rbp[hh] = rb
            shB3 = st["shB"]
            for half in (0, 1):
                hs = slice(half * 512, half * 512 + 512)
                nc.vector.tensor_tensor(
                    out=shB3[:, hs], in0=shB3[:, hs], in1=rbp[1][:, hs], op=ALU.mult
                )
                nc.vector.tensor_tensor(
                    out=otmp[pl][0:64, hs], in0=otmp[pl][0:64, hs],
                    in1=rbp[0][:, hs], op=ALU.mult,
                )

            # projection: per q-tile, K=128 matmuls for pairs 0-2 plus two
            # K=64 matmuls for the unpacked last pair
            for i in range(NQT):
                qsl = slice(i * 128, (i + 1) * 128)
                ps = psum_s.tile([128, 512], F32, name=f"f{i}", tag="ps")
                for p in range(NPAIR - 1):
                    nc.tensor.matmul(
                        ps,
                        lhsT=otmp[p][:, qsl],
                        rhs=wtsp[p],
                        start=(p == 0),
                        stop=False,
                    )
                nc.tensor.matmul(
                    ps, lhsT=shB3[:, qsl], rhs=w3[1],
                    start=False, stop=False,
                )
                nc.tensor.matmul(
                    ps, lhsT=otmp[pl][0:64, qsl], rhs=w3[0],
                    start=False, stop=True,
                )
                nc.vector.tensor_tensor(out=fsb[i], in0=ps, in1=bias_bc, op=ALU.add)
                nc.sync.dma_start(out=out[i * 128 : (i + 1) * 128, :], in_=fsb[i])

    _split_excess_waits(nc)
    return nc


_NC_CACHE = {}


def _get_nc():
    if "nc" not in _NC_CACHE:
        _NC_CACHE["nc"] = _build()
    return _NC_CACHE["nc"]


def kernel(keys, queries, values, W_comb, b_comb, _collect=None):
    from concourse.bass_utils import run_bass_kernel_spmd

    keys = np.ascontiguousarray(keys, dtype=np.float32)
    queries = np.ascontiguousarray(queries, dtype=np.float32)
    values16 = np.ascontiguousarray(values, dtype=np.float16)
    W_comb = np.ascontiguousarray(W_comb, dtype=np.float32)
    b_comb = np.ascontiguousarray(b_comb, dtype=np.float32)

    nc = _get_nc()
    wt_np = np.ascontiguousarray(W_comb.T)
    in_maps = []
    for c in range(NCORES):
        b, half = divmod(c, 2)
        in_maps.append(
            {
                "qt": np.ascontiguousarray(
                    queries[b, half * NQ : (half + 1) * NQ, :].T
                ),
                "kt": np.ascontiguousarray(keys[b].T),
                "v": values16[b],
                "wt": wt_np,
                "bvec": b_comb,
            }
        )
    kwargs = dict(_collect) if _collect else {}
    res = run_bass_kernel_spmd(nc, in_maps, core_ids=list(range(NCORES)), **kwargs)

    full = np.empty((B, N, D), dtype=np.float32)
    for c, r in enumerate(res.results):
        b, half = divmod(c, 2)
        full[b, half * NQ : (half + 1) * NQ, :] = r["out"]
    if _collect is not None:
        return full, res
    return full
